# revision 7
# baseline (speedup 1.0000x reference)
"""MoE layer (top-2 of 8 experts, selection shared across tokens) on 8 TRN2 cores.

Math (faithful to the reference):
    gates = softmax(x @ W_gate + b_gate)          [N, 8]
    idx0  = top-2 expert indices of token 0       [2]
    s     = per-token top-2 gate VALUES (desc)    [N, 2]
    out   = s0 * (x @ W[A] + b[A]) + s1 * (x @ W[B] + b[B])

Strategy: gating + top-2 is 0.2% of the FLOPs -> computed on host.  The bias
term s0*bA + s1*bB is a rank-2 correction (scores @ b_sel) also added on host,
so the device runs only the two weighted matmuls (275 GFLOP), data-parallel
over tokens across 8 cores with replicated expert weights.  Matmuls run in
fp16 (values are small, so fp16 range is safe and its 10-bit mantissa keeps
rel-err ~3e-4), accumulating fp32 in PSUM.  The combined result is written
back in fp16 (adds ~3e-4 rounding, still ~50x under the 2e-2 gate) and
upcast on host.

Schedule notes (from trace analysis):
  - steady-state MM cadence is at the hw floor (512/2.4GHz + ~3ns); all the
    recoverable time is at the edges (head DMA fill, HAM cold clock, tail).
  - ~10 dummy matmuls on an (uninitialized) SBUF tile run during the initial
    DMA fill with no dependencies: they hold the PE busy so the HAM
    clock-gate reaches K=8/8 before the first real matmul.
  - block 0 runs its first two token-slices k-OUTER across 4 m-groups
    (all 8 PSUM banks): each W k-chunk pair is consumed by 8 matmuls
    (1.7us) instead of 2, so the 2-queue DMA feed (~220 GB/s) keeps up
    and the PE is not starved during the cold fill.
  - output DMA rides the two HWDGE queues (sync/scalar), not SWDGE: the
    kernel-tail GpSimd DRAIN walks the SW rings and cost 6.3us when
    outputs used SWDGE.
  - W chunks for the A/B experts share one [128,2,512] tile (fewer tile
    slots -> fewer semaphores -> shorter kernel-tail semaphore-reset storm,
    which is ~33ns per allocated semaphore).
"""

import functools

import numpy as np

import concourse.bass as bass
import concourse.mybir as mybir
import concourse.tile as tile
from concourse import bacc
from concourse.bass_utils import run_bass_kernel_spmd

N_CORES = 8
N, D_IN, D_HID = 16384, 2048, 2048
NT = N // N_CORES            # tokens per core
KP = 128                     # contraction chunk = partition dim
KCH = D_IN // KP             # 16 K-chunks
NB = 512                     # output column block (1 PSUM bank of fp32)
NBLK = D_HID // NB           # 4 output blocks
TQ = 256                     # token slice per x-stream piece
NQ = NT // TQ                # 8 slices
MPQ = TQ // 128              # m-tiles per slice
NWARM = 10                   # HAM warm-up matmuls (cover the DMA head)

F32 = mybir.dt.float32
FP16 = mybir.dt.float16

W_DT = FP16
X_DT = FP16
O_DT = FP16

# Filled by test harness inspection: last BassKernelResults from a run.
LAST_RESULT = None


@functools.lru_cache(maxsize=1)
def _build():
    nc = bacc.Bacc("TRN2", target_bir_lowering=False, debug=False)
    xT = nc.dram_tensor("xT", [D_IN, NT], X_DT, kind="ExternalInput")
    wa = nc.dram_tensor("wa", [D_IN, D_HID], W_DT, kind="ExternalInput")
    wb = nc.dram_tensor("wb", [D_IN, D_HID], W_DT, kind="ExternalInput")
    # per-token scores pre-arranged on host, partition-major:
    # sC[p, m, s] = top2_score[m*128 + p, s]
    sC = nc.dram_tensor("sC", [128, NT // 128, 2], F32, kind="ExternalInput")
    out = nc.dram_tensor("out", [NT, D_HID], O_DT, kind="ExternalOutput")

    MULT = mybir.AluOpType.mult
    ADD = mybir.AluOpType.add

    with tile.TileContext(nc) as tc:
        with (
            tc.tile_pool(name="cst", bufs=1) as cst,
            tc.tile_pool(name="wm", bufs=1) as wm,
            tc.tile_pool(name="wp", bufs=2) as wp,
            tc.tile_pool(name="xp", bufs=3) as xp,
            tc.tile_pool(name="ep", bufs=3) as ep,
            tc.tile_pool(name="ps", bufs=4, space=bass.MemorySpace.PSUM) as ps,
        ):
            # HAM warm-up: a chain of matmuls on a never-written tile, no
            # dependencies at all.  They run during the initial W/x DMA fill
            # and keep the PE busy so the clock-gate is at K=8/8 (2.4 GHz)
            # by the time real matmuls start.  The values are garbage; the
            # target PSUM tile is one rotation slot of the pa tag, never
            # read, and fully overwritten later (start=True clears it).
            wz = wm.tile([KP, NB], X_DT, tag="wz")
            nc.gpsimd.memset(wz[:], 0.0)
            pwt = ps.tile([128, NB], F32, tag="pa")
            for _ in range(NWARM):
                nc.tensor.matmul(pwt[:], wz[:, 0:128], wz[:], start=True, stop=True)

            # constants: 16KB, ~100ns on the sync queue ahead of the streams
            sC_sb = cst.tile([128, NT // 128, 2], F32)
            nc.sync.dma_start(sC_sb[:], sC[:])

            # sync + scalar are the two HWDGE issue queues; W and x are
            # split across both to halve arrival latency.
            def load_x(q):
                x_t = []
                for k in range(KCH):
                    t = xp.tile([KP, TQ], X_DT, tag=f"x{k}")
                    eng = nc.sync if k % 2 == 0 else nc.scalar
                    eng.dma_start(
                        t[:], xT[k * KP:(k + 1) * KP, q * TQ:(q + 1) * TQ]
                    )
                    x_t.append(t)
                return x_t

            def load_w(k, nb_sl):
                # paired tile: [:, 0, :] = expert A chunk, [:, 1, :] = B
                t = wp.tile([KP, 2, NB], W_DT, tag=f"w{k}")
                nc.sync.dma_start(t[:, 0, :], wa[k * KP:(k + 1) * KP, nb_sl])
                nc.scalar.dma_start(t[:, 1, :], wb[k * KP:(k + 1) * KP, nb_sl])
                return t

            def epilogue(pa, pb, mg, nb_sl, last=False):
                s0 = sC_sb[:, mg, 0:1]
                s1 = sC_sb[:, mg, 1:2]
                # out = s0*pa + s1*pb on DVE (each op reads one PSUM input)
                t1 = ep.tile([128, NB], F32, tag="t1")
                nc.vector.tensor_scalar_mul(t1[:], pa[:], s0)
                o = ep.tile([128, NB], O_DT, tag="o")
                nc.vector.scalar_tensor_tensor(
                    o[:], pb[:], s1, t1[:], op0=MULT, op1=ADD
                )
                m_sl = bass.ts(mg, 128)
                if last:
                    # split the final store across both queues to shorten
                    # the kernel tail
                    h = NB // 2
                    c0 = (NBLK - 1) * NB
                    nc.sync.dma_start(out[m_sl, c0:c0 + h], o[:, 0:h])
                    nc.scalar.dma_start(out[m_sl, c0 + h:c0 + NB], o[:, h:NB])
                else:
                    eng = nc.sync if mg % 2 == 0 else nc.scalar
                    eng.dma_start(out[m_sl, nb_sl], o[:])

            for nb in range(NBLK):
                nb_sl = bass.ts(nb, NB)
                w_t = {}
                if nb == 0:
                    # Cold start: the DMA feed (~220 GB/s over 2 queues)
                    # cannot keep up with the k-inner loop's W consumption
                    # (one 256KB chunk-pair per 4 matmuls).  Run the first
                    # two token slices as ONE k-outer super-group over 4
                    # m-groups and all 8 PSUM banks: each W pair feeds 8
                    # matmuls (1.7us), matching the feed rate, and W chunks
                    # are consumed strictly in arrival order.
                    xq = {}
                    for k in range(KCH):
                        t0 = xp.tile([KP, TQ], X_DT, tag=f"x{k}")
                        nc.sync.dma_start(t0[:], xT[k * KP:(k + 1) * KP, 0:TQ])
                        t1s = xp.tile([KP, TQ], X_DT, tag=f"x{k}")
                        nc.scalar.dma_start(t1s[:], xT[k * KP:(k + 1) * KP, TQ:2 * TQ])
                        xq[0, k] = t0
                        xq[1, k] = t1s
                        w_t[k] = load_w(k, nb_sl)
                    pas = [ps.tile([128, NB], F32, tag="pa", name=f"pa_cold{g}")
                           for g in range(4)]
                    pbs = [ps.tile([128, NB], F32, tag="pb", name=f"pb_cold{g}")
                           for g in range(4)]
                    for k in range(KCH):
                        for g in range(4):
                            q, mi = divmod(g, MPQ)
                            xk = xq[q, k][:, bass.ts(mi, 128)]
                            nc.tensor.matmul(
                                pas[g][:], xk, w_t[k][:, 0, :],
                                start=(k == 0), stop=(k == KCH - 1),
                            )
                            nc.tensor.matmul(
                                pbs[g][:], xk, w_t[k][:, 1, :],
                                start=(k == 0), stop=(k == KCH - 1),
                            )
                    for g in range(4):
                        epilogue(pas[g], pbs[g], g, nb_sl)
                    q_start = 2
                else:
                    x_first = load_x(0)
                    for k in range(KCH):
                        w_t[k] = load_w(k, nb_sl)
                    q_start = 0
                for q in range(q_start, NQ):
                    x_t = x_first if (nb != 0 and q == 0) else load_x(q)
                    for mi in range(MPQ):
                        mg = q * MPQ + mi
                        pa = ps.tile([128, NB], F32, tag="pa")
                        pb = ps.tile([128, NB], F32, tag="pb")
                        for k in range(KCH):
                            xk = x_t[k][:, bass.ts(mi, 128)]
                            nc.tensor.matmul(
                                pa[:], xk, w_t[k][:, 0, :],
                                start=(k == 0), stop=(k == KCH - 1),
                            )
                            nc.tensor.matmul(
                                pb[:], xk, w_t[k][:, 1, :],
                                start=(k == 0), stop=(k == KCH - 1),
                            )
                        epilogue(pa, pb, mg, nb_sl,
                                 last=(nb == NBLK - 1 and mg == NQ * MPQ - 1))

    nc.compile()
    return nc


def _host_gating(x, W_gate, b_gate):
    logits = x @ W_gate + b_gate                       # [N, 8] fp32
    m = logits.max(axis=1, keepdims=True)
    e = np.exp(logits - m)
    gates = e / e.sum(axis=1, keepdims=True)
    idx0 = np.argsort(-gates[0], kind="stable")[:2]    # token-0 top-2 experts
    scores = -np.sort(-gates, axis=1)[:, :2]           # per-token top-2 values
    return idx0, np.ascontiguousarray(scores)


def kernel(x, W_experts, b_experts, W_gate, b_gate):
    global LAST_RESULT
    x = np.ascontiguousarray(np.asarray(x, dtype=np.float32))
    W_experts = np.asarray(W_experts, dtype=np.float32)
    b_experts = np.asarray(b_experts, dtype=np.float32)
    W_gate = np.asarray(W_gate, dtype=np.float32)
    b_gate = np.asarray(b_gate, dtype=np.float32)

    idx0, scores = _host_gating(x, W_gate, b_gate)
    w_np_dt = mybir.dt.np(W_DT)
    x_np_dt = mybir.dt.np(X_DT)
    wa = np.ascontiguousarray(W_experts[idx0[0]]).astype(w_np_dt)  # [D_IN, D_HID]
    wb = np.ascontiguousarray(W_experts[idx0[1]]).astype(w_np_dt)

    xT_full = np.ascontiguousarray(x.astype(x_np_dt).T)            # [D_IN, N]

    nc = _build()
    in_maps = []
    for c in range(N_CORES):
        sl = slice(c * NT, (c + 1) * NT)
        in_maps.append(
            {
                "xT": np.ascontiguousarray(xT_full[:, sl]),
                "wa": wa,
                "wb": wb,
                "sC": np.ascontiguousarray(
                    scores[sl].reshape(NT // 128, 128, 2).transpose(1, 0, 2)
                ),
            }
        )

    res = run_bass_kernel_spmd(nc, in_maps, list(range(N_CORES)))
    LAST_RESULT = res
    out = np.concatenate(
        [r["out"] for r in res.results], axis=0
    ).astype(np.float32)
    # bias term s0*bA + s1*bB is a rank-2 correction, added here in fp32
    out += scores @ b_experts[idx0]
    return out


# revision 8
# speedup vs baseline: 1.0117x; 1.0117x over previous
"""MoE layer (top-2 of 8 experts, selection shared across tokens) on 8 TRN2 cores.

Math (faithful to the reference):
    gates = softmax(x @ W_gate + b_gate)          [N, 8]
    idx0  = top-2 expert indices of token 0       [2]
    s     = per-token top-2 gate VALUES (desc)    [N, 2]
    out   = s0 * (x @ W[A] + b[A]) + s1 * (x @ W[B] + b[B])

Strategy: gating + top-2 is 0.2% of the FLOPs -> computed on host.  The bias
term s0*bA + s1*bB is a rank-2 correction (scores @ b_sel) also added on host,
so the device runs only the two weighted matmuls (275 GFLOP), data-parallel
over tokens across 8 cores with replicated expert weights.  Matmuls run in
fp16 (values are small, so fp16 range is safe and its 10-bit mantissa keeps
rel-err ~3e-4), accumulating fp32 in PSUM.  The combined result is written
back in fp16 (adds ~3e-4 rounding, still ~50x under the 2e-2 gate) and
upcast on host.

Schedule notes (from trace analysis):
  - steady-state MM cadence is at the hw floor (512/2.4GHz + ~3ns); all the
    recoverable time is at the edges (head DMA fill, HAM cold clock, tail).
  - ~10 dummy matmuls on an (uninitialized) SBUF tile run during the initial
    DMA fill with no dependencies: they hold the PE busy so the HAM
    clock-gate reaches K=8/8 before the first real matmul.
  - block 0 runs its first two token-slices k-OUTER across 4 m-groups
    (all 8 PSUM banks): each W k-chunk pair is consumed by 8 matmuls
    (1.7us) instead of 2, so the 2-queue DMA feed (~220 GB/s) keeps up
    and the PE is not starved during the cold fill.
  - output DMA rides the two HWDGE queues (sync/scalar), not SWDGE: the
    kernel-tail GpSimd DRAIN walks the SW rings and cost 6.3us when
    outputs used SWDGE.
  - W chunks for the A/B experts share one [128,2,512] tile (fewer tile
    slots -> fewer semaphores -> shorter kernel-tail semaphore-reset storm,
    which is ~33ns per allocated semaphore).
"""

import functools

import numpy as np

import concourse.bass as bass
import concourse.mybir as mybir
import concourse.tile as tile
from concourse import bacc
from concourse.bass_utils import run_bass_kernel_spmd

N_CORES = 8
N, D_IN, D_HID = 16384, 2048, 2048
NT = N // N_CORES            # tokens per core
KP = 128                     # contraction chunk = partition dim
KCH = D_IN // KP             # 16 K-chunks
NB = 512                     # output column block (1 PSUM bank of fp32)
NBLK = D_HID // NB           # 4 output blocks
TQ = 256                     # token slice per x-stream piece
NQ = NT // TQ                # 8 slices
MPQ = TQ // 128              # m-tiles per slice
NWARM = 10                   # HAM warm-up matmuls (cover the DMA head)

F32 = mybir.dt.float32
FP16 = mybir.dt.float16

W_DT = FP16
X_DT = FP16
O_DT = FP16

# Filled by test harness inspection: last BassKernelResults from a run.
LAST_RESULT = None


@functools.lru_cache(maxsize=1)
def _build():
    nc = bacc.Bacc("TRN2", target_bir_lowering=False, debug=False)
    xT = nc.dram_tensor("xT", [D_IN, NT], X_DT, kind="ExternalInput")
    wa = nc.dram_tensor("wa", [D_IN, D_HID], W_DT, kind="ExternalInput")
    wb = nc.dram_tensor("wb", [D_IN, D_HID], W_DT, kind="ExternalInput")
    # per-token scores pre-arranged on host, partition-major:
    # sC[p, m, s] = top2_score[m*128 + p, s]
    sC = nc.dram_tensor("sC", [128, NT // 128, 2], F32, kind="ExternalInput")
    out = nc.dram_tensor("out", [NT, D_HID], O_DT, kind="ExternalOutput")

    MULT = mybir.AluOpType.mult
    ADD = mybir.AluOpType.add

    with tile.TileContext(nc) as tc:
        with (
            tc.tile_pool(name="cst", bufs=1) as cst,
            tc.tile_pool(name="wm", bufs=1) as wm,
            tc.tile_pool(name="wp", bufs=2) as wp,
            tc.tile_pool(name="xp", bufs=4) as xp,
            tc.tile_pool(name="ep", bufs=3) as ep,
            tc.tile_pool(name="ps", bufs=4, space=bass.MemorySpace.PSUM) as ps,
        ):
            # HAM warm-up: a chain of matmuls on a never-written tile, no
            # dependencies at all.  They run during the initial W/x DMA fill
            # and keep the PE busy so the clock-gate is at K=8/8 (2.4 GHz)
            # by the time real matmuls start.  The values are garbage; the
            # target PSUM tile is one rotation slot of the pa tag, never
            # read, and fully overwritten later (start=True clears it).
            wz = wm.tile([KP, NB], X_DT, tag="wz")
            nc.gpsimd.memset(wz[:], 0.0)
            pwt = ps.tile([128, NB], F32, tag="pa")
            for _ in range(NWARM):
                nc.tensor.matmul(pwt[:], wz[:, 0:128], wz[:], start=True, stop=True)

            # constants: 16KB, ~100ns on the sync queue ahead of the streams
            sC_sb = cst.tile([128, NT // 128, 2], F32)
            nc.sync.dma_start(sC_sb[:], sC[:])

            # sync + scalar are the two HWDGE issue queues; W and x are
            # split across both to halve arrival latency.
            def load_x(q):
                x_t = []
                for k in range(KCH):
                    t = xp.tile([KP, TQ], X_DT, tag=f"x{k}")
                    eng = nc.sync if k % 2 == 0 else nc.scalar
                    eng.dma_start(
                        t[:], xT[k * KP:(k + 1) * KP, q * TQ:(q + 1) * TQ]
                    )
                    x_t.append(t)
                return x_t

            def load_w(k, nb_sl):
                # paired tile: [:, 0, :] = expert A chunk, [:, 1, :] = B
                t = wp.tile([KP, 2, NB], W_DT, tag=f"w{k}")
                nc.sync.dma_start(t[:, 0, :], wa[k * KP:(k + 1) * KP, nb_sl])
                nc.scalar.dma_start(t[:, 1, :], wb[k * KP:(k + 1) * KP, nb_sl])
                return t

            def epilogue(pa, pb, mg, nb_sl, last=False):
                s0 = sC_sb[:, mg, 0:1]
                s1 = sC_sb[:, mg, 1:2]
                # out = s0*pa + s1*pb on DVE (each op reads one PSUM input)
                t1 = ep.tile([128, NB], F32, tag="t1")
                nc.vector.tensor_scalar_mul(t1[:], pa[:], s0)
                o = ep.tile([128, NB], O_DT, tag="o")
                nc.vector.scalar_tensor_tensor(
                    o[:], pb[:], s1, t1[:], op0=MULT, op1=ADD
                )
                m_sl = bass.ts(mg, 128)
                if last:
                    # split the final store across both queues to shorten
                    # the kernel tail
                    h = NB // 2
                    c0 = (NBLK - 1) * NB
                    nc.sync.dma_start(out[m_sl, c0:c0 + h], o[:, 0:h])
                    nc.scalar.dma_start(out[m_sl, c0 + h:c0 + NB], o[:, h:NB])
                else:
                    eng = nc.sync if mg % 2 == 0 else nc.scalar
                    eng.dma_start(out[m_sl, nb_sl], o[:])

            for nb in range(NBLK):
                nb_sl = bass.ts(nb, NB)
                w_t = {}
                if nb == 0:
                    # Cold start: the DMA feed (~220 GB/s over 2 queues)
                    # cannot keep up with the k-inner loop's W consumption
                    # (one 256KB chunk-pair per 4 matmuls).  Run the first
                    # two token slices as ONE k-outer super-group over 4
                    # m-groups and all 8 PSUM banks: each W pair feeds 8
                    # matmuls (1.7us), matching the feed rate, and W chunks
                    # are consumed strictly in arrival order.
                    xq = {}
                    for k in range(KCH):
                        t0 = xp.tile([KP, TQ], X_DT, tag=f"x{k}")
                        nc.sync.dma_start(t0[:], xT[k * KP:(k + 1) * KP, 0:TQ])
                        t1s = xp.tile([KP, TQ], X_DT, tag=f"x{k}")
                        nc.scalar.dma_start(t1s[:], xT[k * KP:(k + 1) * KP, TQ:2 * TQ])
                        xq[0, k] = t0
                        xq[1, k] = t1s
                        w_t[k] = load_w(k, nb_sl)
                    pas = [ps.tile([128, NB], F32, tag="pa", name=f"pa_cold{g}")
                           for g in range(4)]
                    pbs = [ps.tile([128, NB], F32, tag="pb", name=f"pb_cold{g}")
                           for g in range(4)]
                    for k in range(KCH):
                        for g in range(4):
                            q, mi = divmod(g, MPQ)
                            xk = xq[q, k][:, bass.ts(mi, 128)]
                            nc.tensor.matmul(
                                pas[g][:], xk, w_t[k][:, 0, :],
                                start=(k == 0), stop=(k == KCH - 1),
                            )
                            nc.tensor.matmul(
                                pbs[g][:], xk, w_t[k][:, 1, :],
                                start=(k == 0), stop=(k == KCH - 1),
                            )
                    for g in range(4):
                        epilogue(pas[g], pbs[g], g, nb_sl)
                    q_start = 2
                else:
                    x_first = load_x(0)
                    for k in range(KCH):
                        w_t[k] = load_w(k, nb_sl)
                    q_start = 0
                for q in range(q_start, NQ):
                    x_t = x_first if (nb != 0 and q == 0) else load_x(q)
                    for mi in range(MPQ):
                        mg = q * MPQ + mi
                        pa = ps.tile([128, NB], F32, tag="pa")
                        pb = ps.tile([128, NB], F32, tag="pb")
                        for k in range(KCH):
                            xk = x_t[k][:, bass.ts(mi, 128)]
                            nc.tensor.matmul(
                                pa[:], xk, w_t[k][:, 0, :],
                                start=(k == 0), stop=(k == KCH - 1),
                            )
                            nc.tensor.matmul(
                                pb[:], xk, w_t[k][:, 1, :],
                                start=(k == 0), stop=(k == KCH - 1),
                            )
                        epilogue(pa, pb, mg, nb_sl,
                                 last=(nb == NBLK - 1 and mg == NQ * MPQ - 1))

    nc.compile()
    return nc


def _host_gating(x, W_gate, b_gate):
    logits = x @ W_gate + b_gate                       # [N, 8] fp32
    m = logits.max(axis=1, keepdims=True)
    e = np.exp(logits - m)
    gates = e / e.sum(axis=1, keepdims=True)
    idx0 = np.argsort(-gates[0], kind="stable")[:2]    # token-0 top-2 experts
    scores = -np.sort(-gates, axis=1)[:, :2]           # per-token top-2 values
    return idx0, np.ascontiguousarray(scores)


def kernel(x, W_experts, b_experts, W_gate, b_gate):
    global LAST_RESULT
    x = np.ascontiguousarray(np.asarray(x, dtype=np.float32))
    W_experts = np.asarray(W_experts, dtype=np.float32)
    b_experts = np.asarray(b_experts, dtype=np.float32)
    W_gate = np.asarray(W_gate, dtype=np.float32)
    b_gate = np.asarray(b_gate, dtype=np.float32)

    idx0, scores = _host_gating(x, W_gate, b_gate)
    w_np_dt = mybir.dt.np(W_DT)
    x_np_dt = mybir.dt.np(X_DT)
    wa = np.ascontiguousarray(W_experts[idx0[0]]).astype(w_np_dt)  # [D_IN, D_HID]
    wb = np.ascontiguousarray(W_experts[idx0[1]]).astype(w_np_dt)

    xT_full = np.ascontiguousarray(x.astype(x_np_dt).T)            # [D_IN, N]

    nc = _build()
    in_maps = []
    for c in range(N_CORES):
        sl = slice(c * NT, (c + 1) * NT)
        in_maps.append(
            {
                "xT": np.ascontiguousarray(xT_full[:, sl]),
                "wa": wa,
                "wb": wb,
                "sC": np.ascontiguousarray(
                    scores[sl].reshape(NT // 128, 128, 2).transpose(1, 0, 2)
                ),
            }
        )

    res = run_bass_kernel_spmd(nc, in_maps, list(range(N_CORES)))
    LAST_RESULT = res
    out = np.concatenate(
        [r["out"] for r in res.results], axis=0
    ).astype(np.float32)
    # bias term s0*bA + s1*bB is a rank-2 correction, added here in fp32
    out += scores @ b_experts[idx0]
    return out


# revision 10
# speedup vs baseline: 1.0213x; 1.0095x over previous
"""MoE layer (top-2 of 8 experts, selection shared across tokens) on 8 TRN2 cores.

Math (faithful to the reference):
    gates = softmax(x @ W_gate + b_gate)          [N, 8]
    idx0  = top-2 expert indices of token 0       [2]
    s     = per-token top-2 gate VALUES (desc)    [N, 2]
    out   = s0 * (x @ W[A] + b[A]) + s1 * (x @ W[B] + b[B])

Strategy: gating + top-2 is 0.2% of the FLOPs -> computed on host.  The bias
term s0*bA + s1*bB is a rank-2 correction (scores @ b_sel) also added on host,
so the device runs only the two weighted matmuls (275 GFLOP), data-parallel
over tokens across 8 cores with replicated expert weights.  Matmuls run in
fp16 (values are small, so fp16 range is safe and its 10-bit mantissa keeps
rel-err ~3e-4), accumulating fp32 in PSUM.  The combined result is written
back in fp16 (adds ~3e-4 rounding, still ~50x under the 2e-2 gate) and
upcast on host.

Schedule notes (from trace analysis):
  - steady-state MM cadence is at the hw floor (512/2.4GHz + ~3ns); all the
    recoverable time is at the edges (head DMA fill, HAM cold clock, tail).
  - ~10 dummy matmuls on an (uninitialized) SBUF tile run during the initial
    DMA fill with no dependencies: they hold the PE busy so the HAM
    clock-gate reaches K=8/8 before the first real matmul.
  - block 0 runs its first two token-slices k-OUTER across 4 m-groups
    (all 8 PSUM banks): each W k-chunk pair is consumed by 8 matmuls
    (1.7us) instead of 2, so the 2-queue DMA feed (~220 GB/s) keeps up
    and the PE is not starved during the cold fill.
  - output DMA rides the two HWDGE queues (sync/scalar), not SWDGE: the
    kernel-tail GpSimd DRAIN walks the SW rings and cost 6.3us when
    outputs used SWDGE.
  - W chunks for the A/B experts share one [128,2,512] tile (fewer tile
    slots -> fewer semaphores -> shorter kernel-tail semaphore-reset storm,
    which is ~33ns per allocated semaphore).
"""

import functools

import numpy as np

import concourse.bass as bass
import concourse.mybir as mybir
import concourse.tile as tile
from concourse import bacc
from concourse.bass_utils import run_bass_kernel_spmd

N_CORES = 8
N, D_IN, D_HID = 16384, 2048, 2048
NT = N // N_CORES            # tokens per core
KP = 128                     # contraction chunk = partition dim
KCH = D_IN // KP             # 16 K-chunks
NB = 512                     # output column block (1 PSUM bank of fp32)
NBLK = D_HID // NB           # 4 output blocks
TQ = 256                     # token slice per x-stream piece
NQ = NT // TQ                # 8 slices
MPQ = TQ // 128              # m-tiles per slice
NWARM = 10                   # HAM warm-up matmuls (cover the DMA head)

F32 = mybir.dt.float32
FP16 = mybir.dt.float16

W_DT = FP16
X_DT = FP16
O_DT = FP16

# Filled by test harness inspection: last BassKernelResults from a run.
LAST_RESULT = None


@functools.lru_cache(maxsize=1)
def _build():
    nc = bacc.Bacc("TRN2", target_bir_lowering=False, debug=False)
    xT = nc.dram_tensor("xT", [D_IN, NT], X_DT, kind="ExternalInput")
    wa = nc.dram_tensor("wa", [D_IN, D_HID], W_DT, kind="ExternalInput")
    wb = nc.dram_tensor("wb", [D_IN, D_HID], W_DT, kind="ExternalInput")
    # per-token scores pre-arranged on host, partition-major:
    # sC[p, m, s] = top2_score[m*128 + p, s]
    sC = nc.dram_tensor("sC", [128, NT // 128, 2], F32, kind="ExternalInput")
    out = nc.dram_tensor("out", [NT, D_HID], O_DT, kind="ExternalOutput")

    MULT = mybir.AluOpType.mult
    ADD = mybir.AluOpType.add

    with tile.TileContext(nc) as tc:
        with (
            tc.tile_pool(name="cst", bufs=1) as cst,
            tc.tile_pool(name="wm", bufs=1) as wm,
            tc.tile_pool(name="wp", bufs=2) as wp,
            tc.tile_pool(name="xp", bufs=4) as xp,
            tc.tile_pool(name="ep", bufs=6) as ep,
            tc.tile_pool(name="ps", bufs=4, space=bass.MemorySpace.PSUM) as ps,
        ):
            # HAM warm-up: a chain of matmuls on a never-written tile, no
            # dependencies at all.  They run during the initial W/x DMA fill
            # and keep the PE busy so the clock-gate is at K=8/8 (2.4 GHz)
            # by the time real matmuls start.  The values are garbage; the
            # target PSUM tile is one rotation slot of the pa tag, never
            # read, and fully overwritten later (start=True clears it).
            wz = wm.tile([KP, NB], X_DT, tag="wz")
            nc.gpsimd.memset(wz[:], 0.0)
            pwt = ps.tile([128, NB], F32, tag="pa")
            for _ in range(NWARM):
                nc.tensor.matmul(pwt[:], wz[:, 0:128], wz[:], start=True, stop=True)

            # constants: 16KB, ~100ns on the sync queue ahead of the streams
            sC_sb = cst.tile([128, NT // 128, 2], F32)
            nc.sync.dma_start(sC_sb[:], sC[:])

            # sync + scalar are the two HWDGE issue queues; W and x are
            # split across both to halve arrival latency.
            def load_x(q):
                x_t = []
                for k in range(KCH):
                    t = xp.tile([KP, TQ], X_DT, tag=f"x{k}")
                    eng = nc.sync if k % 2 == 0 else nc.scalar
                    eng.dma_start(
                        t[:], xT[k * KP:(k + 1) * KP, q * TQ:(q + 1) * TQ]
                    )
                    x_t.append(t)
                return x_t

            def load_w(k, nb_sl):
                # paired tile: [:, 0, :] = expert A chunk, [:, 1, :] = B
                t = wp.tile([KP, 2, NB], W_DT, tag=f"w{k}")
                nc.sync.dma_start(t[:, 0, :], wa[k * KP:(k + 1) * KP, nb_sl])
                nc.scalar.dma_start(t[:, 1, :], wb[k * KP:(k + 1) * KP, nb_sl])
                return t

            def epilogue(pa, pb, mg, nb_sl, last=False):
                s0 = sC_sb[:, mg, 0:1]
                s1 = sC_sb[:, mg, 1:2]
                # out = s0*pa + s1*pb on DVE (each op reads one PSUM input)
                t1 = ep.tile([128, NB], F32, tag="t1")
                nc.vector.tensor_scalar_mul(t1[:], pa[:], s0)
                o = ep.tile([128, NB], O_DT, tag="o")
                nc.vector.scalar_tensor_tensor(
                    o[:], pb[:], s1, t1[:], op0=MULT, op1=ADD
                )
                m_sl = bass.ts(mg, 128)
                if last:
                    # split the final store across both queues to shorten
                    # the kernel tail
                    h = NB // 2
                    c0 = (NBLK - 1) * NB
                    nc.sync.dma_start(out[m_sl, c0:c0 + h], o[:, 0:h])
                    nc.scalar.dma_start(out[m_sl, c0 + h:c0 + NB], o[:, h:NB])
                else:
                    eng = nc.sync if mg % 2 == 0 else nc.scalar
                    eng.dma_start(out[m_sl, nb_sl], o[:])

            for nb in range(NBLK):
                nb_sl = bass.ts(nb, NB)
                w_t = {}
                if nb == 0:
                    # Cold start: the DMA feed (~220 GB/s over 2 queues)
                    # cannot keep up with the k-inner loop's W consumption
                    # (one 256KB chunk-pair per 4 matmuls).  Run the first
                    # two token slices as ONE k-outer super-group over 4
                    # m-groups and all 8 PSUM banks: each W pair feeds 8
                    # matmuls (1.7us), matching the feed rate, and W chunks
                    # are consumed strictly in arrival order.
                    xq = {}
                    for k in range(KCH):
                        t0 = xp.tile([KP, TQ], X_DT, tag=f"x{k}")
                        nc.sync.dma_start(t0[:], xT[k * KP:(k + 1) * KP, 0:TQ])
                        t1s = xp.tile([KP, TQ], X_DT, tag=f"x{k}")
                        nc.scalar.dma_start(t1s[:], xT[k * KP:(k + 1) * KP, TQ:2 * TQ])
                        xq[0, k] = t0
                        xq[1, k] = t1s
                        w_t[k] = load_w(k, nb_sl)
                    pas = [ps.tile([128, NB], F32, tag="pa", name=f"pa_cold{g}")
                           for g in range(4)]
                    pbs = [ps.tile([128, NB], F32, tag="pb", name=f"pb_cold{g}")
                           for g in range(4)]
                    for k in range(KCH):
                        for g in range(4):
                            q, mi = divmod(g, MPQ)
                            xk = xq[q, k][:, bass.ts(mi, 128)]
                            nc.tensor.matmul(
                                pas[g][:], xk, w_t[k][:, 0, :],
                                start=(k == 0), stop=(k == KCH - 1),
                            )
                            nc.tensor.matmul(
                                pbs[g][:], xk, w_t[k][:, 1, :],
                                start=(k == 0), stop=(k == KCH - 1),
                            )
                    # issue the next two x-slices BEFORE the super-group
                    # epilogues: the out-DMAs wait on the DVE epilogue and
                    # would head-of-line-block the (FIFO) HWDGE queues,
                    # starving the q2/q3 matmuls of x data.
                    x_pre = {2: load_x(2), 3: load_x(3)}
                    for g in range(4):
                        epilogue(pas[g], pbs[g], g, nb_sl)
                    q_start = 2
                else:
                    x_pre = {0: load_x(0)}
                    for k in range(KCH):
                        w_t[k] = load_w(k, nb_sl)
                    q_start = 0
                for q in range(q_start, NQ):
                    x_t = x_pre[q] if q in x_pre else load_x(q)
                    for mi in range(MPQ):
                        mg = q * MPQ + mi
                        pa = ps.tile([128, NB], F32, tag="pa")
                        pb = ps.tile([128, NB], F32, tag="pb")
                        for k in range(KCH):
                            xk = x_t[k][:, bass.ts(mi, 128)]
                            nc.tensor.matmul(
                                pa[:], xk, w_t[k][:, 0, :],
                                start=(k == 0), stop=(k == KCH - 1),
                            )
                            nc.tensor.matmul(
                                pb[:], xk, w_t[k][:, 1, :],
                                start=(k == 0), stop=(k == KCH - 1),
                            )
                        epilogue(pa, pb, mg, nb_sl,
                                 last=(nb == NBLK - 1 and mg == NQ * MPQ - 1))

    nc.compile()
    return nc


def _host_gating(x, W_gate, b_gate):
    logits = x @ W_gate + b_gate                       # [N, 8] fp32
    m = logits.max(axis=1, keepdims=True)
    e = np.exp(logits - m)
    gates = e / e.sum(axis=1, keepdims=True)
    idx0 = np.argsort(-gates[0], kind="stable")[:2]    # token-0 top-2 experts
    scores = -np.sort(-gates, axis=1)[:, :2]           # per-token top-2 values
    return idx0, np.ascontiguousarray(scores)


def kernel(x, W_experts, b_experts, W_gate, b_gate):
    global LAST_RESULT
    x = np.ascontiguousarray(np.asarray(x, dtype=np.float32))
    W_experts = np.asarray(W_experts, dtype=np.float32)
    b_experts = np.asarray(b_experts, dtype=np.float32)
    W_gate = np.asarray(W_gate, dtype=np.float32)
    b_gate = np.asarray(b_gate, dtype=np.float32)

    idx0, scores = _host_gating(x, W_gate, b_gate)
    w_np_dt = mybir.dt.np(W_DT)
    x_np_dt = mybir.dt.np(X_DT)
    wa = np.ascontiguousarray(W_experts[idx0[0]]).astype(w_np_dt)  # [D_IN, D_HID]
    wb = np.ascontiguousarray(W_experts[idx0[1]]).astype(w_np_dt)

    xT_full = np.ascontiguousarray(x.astype(x_np_dt).T)            # [D_IN, N]

    nc = _build()
    in_maps = []
    for c in range(N_CORES):
        sl = slice(c * NT, (c + 1) * NT)
        in_maps.append(
            {
                "xT": np.ascontiguousarray(xT_full[:, sl]),
                "wa": wa,
                "wb": wb,
                "sC": np.ascontiguousarray(
                    scores[sl].reshape(NT // 128, 128, 2).transpose(1, 0, 2)
                ),
            }
        )

    res = run_bass_kernel_spmd(nc, in_maps, list(range(N_CORES)))
    LAST_RESULT = res
    out = np.concatenate(
        [r["out"] for r in res.results], axis=0
    ).astype(np.float32)
    # bias term s0*bA + s1*bB is a rank-2 correction, added here in fp32
    out += scores @ b_experts[idx0]
    return out
